# revision 58
# baseline (speedup 1.0000x reference)
"""Trainium2 Bass kernel for nn_LSTMActor: trunk GEMM -> LayerNorm -> Tanh ->
LSTM (16 steps, constant input) -> MLP head -> tanh.

Sharding: data-parallel over batch B=2048 across 8 cores (256 rows each);
all weights replicated. Everything after the trunk runs in a transposed
layout (feature dim on partitions) so no per-step transposes are needed.

v2: fp16 operands everywhere (same PE speed as bf16, 8x the mantissa);
host-side pre-rearranged weight layouts so trunk DMA reads are 8KB
contiguous per partition; LayerNorm/transpose fused per b-tile right
after the trunk; pre-projection emitted in j-major order so the first
LSTM step starts while pre is still being computed; W2 head runs
transposed ([6,BS] output, bias folded into the tanh activation) with
per-step output DMA.
"""

import numpy as np

import concourse.bass as bass
import concourse.tile as tile
from concourse import mybir, bacc
from concourse import bass_utils
from concourse.masks import make_identity

F16H = np.float16
F32 = mybir.dt.float32
F16 = mybir.dt.float16

B, R, Fd, H, A, T = 2048, 39200, 1024, 1024, 6, 16
NC_ = 8
BS = B // NC_          # 256 rows per core
NB = BS // 128         # 2 b-tiles per core
KT = 128               # contraction tile
RP = ((R + KT - 1) // KT) * KT   # 39296, padded R
NK = RP // KT          # 307 K-tiles for trunk
KH = H // 128          # 8 K-tiles for H-dim GEMMs
M4 = 4 * H // 128      # 32 M-tiles of gates
H2 = H // 2            # 512
NH2 = H2 // 128        # 4
KG = 4                 # trunk K-tiles per DMA batch

_CACHE = {}


def _build():
    nc = bacc.Bacc("TRN2", target_bir_lowering=False, debug=False)

    obsT_d = nc.dram_tensor("obsT", [128, NK, BS], F16, kind="ExternalInput")
    wtr_d = nc.dram_tensor("wtr", [128, NK, Fd], F16, kind="ExternalInput")
    wih_d = nc.dram_tensor("wih", [M4, 128, KH * 128], F16, kind="ExternalInput")
    whh_d = nc.dram_tensor("whh", [128, KH, 4 * H], F16, kind="ExternalInput")
    w1_d = nc.dram_tensor("w1", [128, KH, H2], F16, kind="ExternalInput")
    w2_d = nc.dram_tensor("w2", [128, NH2, A], F16, kind="ExternalInput")
    gam_d = nc.dram_tensor("gam", [Fd], F32, kind="ExternalInput")
    bet_d = nc.dram_tensor("bet", [Fd], F32, kind="ExternalInput")
    bsum_d = nc.dram_tensor("bsum", [128, M4], F32, kind="ExternalInput")
    b1_d = nc.dram_tensor("b1", [128, NH2], F32, kind="ExternalInput")
    b2_d = nc.dram_tensor("b2", [A], F32, kind="ExternalInput")
    mu_d = nc.dram_tensor("mu", [A, T * BS], F32, kind="ExternalOutput")

    AF = mybir.ActivationFunctionType

    def bc(ap1d, p=128):
        return bass.AP(tensor=ap1d.tensor, offset=ap1d.offset,
                       ap=[[0, p]] + [list(x) for x in ap1d.ap])

    with tile.TileContext(nc) as tc:
        with (
            tc.tile_pool(name="const", bufs=1) as cst,
            tc.tile_pool(name="state", bufs=1) as st,
            tc.tile_pool(name="wstream", bufs=2) as ws,
            tc.tile_pool(name="work", bufs=1) as wk,
            tc.tile_pool(name="acts", bufs=2) as ac,
        ):
            # ---- resident constants ----
            ident = cst.tile([128, 128], F16)
            make_identity(nc, ident)
            whh_sb = cst.tile([128, KH, 4 * H], F16)      # 64KB/part
            w1_sb = cst.tile([128, KH, H2], F16)          # 8KB/part
            w2_sb = cst.tile([128, NH2, A], F16)          # [128,4,6]
            # b_trunk is folded into the padded trunk contraction row
            # (wtr row R = btr, obsT row R = 1), so btr_b is just zeros:
            # memset on the idle Pool engine instead of a 512KB broadcast
            # DMA in the DMA-bound trunk stream. The LN add of zero keeps
            # the proven pipeline shape (psx last-read stays at the adds).
            btr_b = cst.tile([128, Fd], F32)
            nc.gpsimd.memset(btr_b, 0.0)
            gam_b = cst.tile([128, Fd], F32)
            bet_b = cst.tile([128, Fd], F32)
            bsum_sb = cst.tile([128, M4], F32)            # col m = bsum[m*128+p]
            b1_sb = cst.tile([128, NH2], F32)             # [128,4]
            b2t = cst.tile([A, 1], F32)                   # per-partition bias
            eps_t = cst.tile([128, 1], F32)
            nc.vector.memset(eps_t, 1e-5)

            # ---- persistent state ----
            xT = st.tile([128, KH, BS], F16)              # x^T  [Fd, BS]
            preT = st.tile([128, M4, BS], F16)            # pre^T [4H, BS]
            c_st = st.tile([128, KH, BS], F32)            # c^T  [H, BS]
            hT = [st.tile([128, KH, BS], F16, name=f"hT{i}", tag=f"h{i}")
                  for i in range(2)]                      # ping-pong
            muT = st.tile([A, T, BS], F32)                # [6,16,256]

            # ================= Phase 1: trunk GEMM =================
            with tc.tile_pool(name="ps_trunk", bufs=1, space="PSUM") as pst:
                psx = pst.tile([128, NB, Fd], F32)        # 8KB/part = 4 banks
                # small first groups so the first matmul starts ASAP
                kgs, kg = [], 0
                for sz in (1, 1, 2):
                    kgs.append((kg, sz))
                    kg += sz
                while kg < NK:
                    kgs.append((kg, min(KG, NK - kg)))
                    kg += kgs[-1][1]
                for gi, (kg, kn) in enumerate(kgs):
                    wt = ws.tile([128, KG, Fd], F16, tag="wtr", bufs=3)
                    ot = ws.tile([128, KG, BS], F16, tag="obsT", bufs=4)
                    nc.sync.dma_start(ot[:, :kn, :], obsT_d.ap()[:, kg : kg + kn, :])
                    nc.sync.dma_start(wt[:, :kn, :], wtr_d.ap()[:, kg : kg + kn, :])
                    # broadcast biases ride the DMA pipe-fill window, split
                    # so each 1.5us bubble is absorbed by the tile buffers
                    if gi == 2:
                        nc.sync.dma_start(gam_b, bc(gam_d.ap()))
                    elif gi == 6:
                        nc.sync.dma_start(bet_b, bc(bet_d.ap()))

                    for kk in range(kn):
                        k = kg + kk
                        for b in range(NB):
                            lhsT = ot[:, kk, b * 128 : (b + 1) * 128]
                            for n in range(2):
                                nc.tensor.matmul(
                                    psx[:, b, n * 512 : (n + 1) * 512],
                                    lhsT,
                                    wt[:, kk, n * 512 : (n + 1) * 512],
                                    start=(k == 0),
                                    stop=(k == NK - 1),
                                )

                # small constants: DMA is free once the trunk stream drains
                nc.sync.dma_start(bsum_sb, bsum_d.ap())
                nc.sync.dma_start(b1_sb, b1_d.ap())
                nc.sync.dma_start(
                    b2t, bass.AP(tensor=b2_d.ap().tensor, offset=b2_d.ap().offset,
                                 ap=[[1, A], [0, 1]]))

                # ==== Phase 2: LayerNorm + tanh + transpose, b-interleaved ====
                # software-pipelined so b1's chain isn't queued behind all of
                # b0's normalize work and pre can start on early f-tiles
                with tc.tile_pool(name="ps_tr", bufs=4, space="PSUM") as ptr:
                    xs_t, stats_t, mv_t, rstd_t, xa_t = [], [], [], [], []
                    for b in range(NB):
                        xs = wk.tile([128, Fd], F32, tag="xs", bufs=2)
                        stats = wk.tile([128, 2, 6], F32, tag="stats", bufs=2)
                        for s in range(2):
                            nc.vector.tensor_add(
                                xs[:, s * 512 : (s + 1) * 512],
                                psx[:, b, s * 512 : (s + 1) * 512],
                                btr_b[:, s * 512 : (s + 1) * 512],
                            )
                            nc.vector.bn_stats(
                                out=stats[:, s, :],
                                in_=xs[:, s * 512 : (s + 1) * 512],
                            )
                        mv = wk.tile([128, 2], F32, tag="mv", bufs=2)
                        nc.vector.bn_aggr(out=mv, in_=stats)
                        rstd = wk.tile([128, 1], F32, tag="rstd", bufs=2)
                        nc.scalar.activation(
                            out=rstd, in_=mv[:, 1:2], func=AF.Sqrt, bias=eps_t,
                            scale=1.0,
                        )
                        xs_t.append(xs)
                        stats_t.append(stats)
                        mv_t.append(mv)
                        rstd_t.append(rstd)
                        xa_t.append(
                            wk.tile([128, Fd], F16, tag="xa", bufs=2,
                                    name=f"xa{b}"))
                    for b in range(NB):
                        nc.vector.reciprocal(out=rstd_t[b], in_=rstd_t[b])
                    for c in range(4):
                        cs = slice(c * 256, (c + 1) * 256)
                        for b in range(NB):
                            xs, xa = xs_t[b], xa_t[b]
                            nc.vector.scalar_tensor_tensor(
                                out=xs[:, cs], in0=xs[:, cs],
                                scalar=mv_t[b][:, 0:1], in1=gam_b[:, cs],
                                op0=mybir.AluOpType.subtract,
                                op1=mybir.AluOpType.mult,
                            )
                            nc.vector.scalar_tensor_tensor(
                                out=xs[:, cs], in0=xs[:, cs], scalar=rstd_t[b],
                                in1=bet_b[:, cs],
                                op0=mybir.AluOpType.mult, op1=mybir.AluOpType.add,
                            )
                            nc.scalar.activation(out=xa[:, cs], in_=xs[:, cs],
                                                 func=AF.Tanh)
                            for fc in range(2):
                                f = c * 2 + fc
                                pt = ptr.tile([128, 128], F16, tag="tr")
                                nc.tensor.transpose(
                                    pt, xa[:, f * 128 : (f + 1) * 128], ident
                                )
                                nc.scalar.activation(
                                    out=xT[:, f, b * 128 : (b + 1) * 128],
                                    in_=pt, func=AF.Copy,
                                )

            # ==== Phase 3: pre^T = W_ih^T x^T + bsum (j-major order) ====
            with tc.tile_pool(name="ps_pre", bufs=2, space="PSUM") as ppr:
                for idx, m in enumerate(
                    m for j in range(KH) for m in (j, 8 + j, 16 + j, 24 + j)
                ):
                    wm = ws.tile([128, KH, 128], F16, tag="wih", bufs=6)
                    nc.sync.dma_start(
                        wm, wih_d.ap()[m].rearrange("p (k j) -> p k j", j=128))
                    # t=1's first gate chain touches ALL whh k-slices within
                    # ~3.4us (k rotates (kk+j)%8), so every whh slice must be
                    # resident by pre-end: full interleave, w1 near the end
                    if idx == 0:
                        nc.sync.dma_start(w2_sb, w2_d.ap())
                    if idx % 4 == 0:
                        k8 = idx // 4
                        nc.sync.dma_start(whh_sb[:, k8, :], whh_d.ap()[:, k8, :])
                    if idx == 26:
                        nc.sync.dma_start(w1_sb, w1_d.ap())
                    ps = ppr.tile([128, BS], F32, tag="pre")
                    for k in range(KH):
                        nc.tensor.matmul(
                            ps, wm[:, k, :], xT[:, k, :],
                            start=(k == 0), stop=(k == KH - 1),
                        )
                    nc.vector.tensor_scalar_add(
                        preT[:, m, :], ps, bsum_sb[:, m : m + 1]
                    )

            # ============ Phase 4: LSTM steps ============
            with (
                tc.tile_pool(name="ps_g", bufs=5, space="PSUM") as psg,
                tc.tile_pool(name="ps_m", bufs=2, space="PSUM") as psm,
                tc.tile_pool(name="ps_w2", bufs=1, space="PSUM") as psw,
            ):
                relu1T = st.tile([128, NH2, BS], F16)

                def cell_update(j, si, sf, tg, so, first):
                    """c[j] = sf*c[j] + si*tg ; h[j] = so*tanh(c[j]) -> h_new."""
                    if first:
                        nc.vector.tensor_mul(c_st[:, j, :], si, tg)
                    else:
                        t1 = ac.tile([128, BS], F32, tag="t1")
                        nc.vector.tensor_mul(t1, si, tg)
                        nc.vector.tensor_mul(c_st[:, j, :], c_st[:, j, :], sf)
                        nc.vector.tensor_add(c_st[:, j, :], c_st[:, j, :], t1)
                    tcn = ac.tile([128, BS], F16, tag="tc")
                    nc.scalar.activation(out=tcn, in_=c_st[:, j, :], func=AF.Tanh)
                    nc.vector.tensor_mul(h_new[:, j, :], so, tcn)

                def mlp_head(t, h_cur):
                    for m in range(NH2):
                        ps = psm.tile([128, BS], F32, tag="m1")
                        for k in range(KH):
                            nc.tensor.matmul(
                                ps, w1_sb[:, k, m * 128 : (m + 1) * 128],
                                h_cur[:, k, :],
                                start=(k == 0), stop=(k == KH - 1),
                            )
                        nc.scalar.activation(
                            out=relu1T[:, m, :], in_=ps, func=AF.Relu,
                            bias=b1_sb[:, m : m + 1], scale=1.0,
                        )
                    ps2 = psw.tile([128, BS], F32, tag="w2")
                    for k2 in range(NH2):
                        nc.tensor.matmul(
                            ps2[:A, :], w2_sb[:, k2, :], relu1T[:, k2, :],
                            start=(k2 == 0), stop=(k2 == NH2 - 1),
                        )
                    nc.scalar.activation(
                        out=muT[:, t, :], in_=ps2[:A, :], func=AF.Tanh, bias=b2t,
                        scale=1.0,
                    )
                    nc.sync.dma_start(mu_d.ap().rearrange(
                        "a (t b) -> a t b", t=T)[:, t, :], muT[:, t, :])

                # ---- step 0: h0 = c0 = 0 -> gates = pre ----
                h_new = hT[0]
                for j in range(KH):
                    si = ac.tile([128, BS], F16, tag="a0")
                    tg = ac.tile([128, BS], F16, tag="a2")
                    so = ac.tile([128, BS], F16, tag="a3")
                    nc.scalar.activation(out=si, in_=preT[:, j, :], func=AF.Sigmoid)
                    nc.scalar.activation(out=tg, in_=preT[:, 16 + j, :], func=AF.Tanh)
                    nc.scalar.activation(out=so, in_=preT[:, 24 + j, :],
                                         func=AF.Sigmoid)
                    cell_update(j, si, None, tg, so, first=True)
                mlp_head(0, hT[0])

                # ---- steps 1..15 ----
                for t in range(1, T):
                    h_cur = hT[(t + 1) % 2]
                    h_new = hT[t % 2]
                    for j in range(KH):
                        acts = {}
                        for q in range(4):
                            m = 8 * q + j
                            ps = psg.tile([128, BS], F32, tag="g")
                            for kk in range(KH):
                                k = (kk + j) % KH
                                nc.tensor.matmul(
                                    ps,
                                    whh_sb[:, k, m * 128 : (m + 1) * 128],
                                    h_cur[:, k, :],
                                    start=(kk == 0), stop=(kk == KH - 1),
                                )
                            tmp = ac.tile([128, BS], F16, tag=f"q{q}")
                            nc.vector.tensor_add(tmp, ps, preT[:, m, :])
                            out_a = ac.tile([128, BS], F16, tag=f"a{q}")
                            nc.scalar.activation(
                                out=out_a, in_=tmp,
                                func=AF.Tanh if q == 2 else AF.Sigmoid,
                            )
                            acts[q] = out_a
                        cell_update(j, acts[0], acts[1], acts[2], acts[3],
                                    first=False)
                    mlp_head(t, h_new)

    nc.compile()
    return nc


def kernel(**inputs):
    obs = np.asarray(inputs["obs"], np.float32)
    W_trunk = np.asarray(inputs["W_trunk"], np.float32)
    b_trunk = np.asarray(inputs["b_trunk"], np.float32)
    gamma = np.asarray(inputs["gamma"], np.float32)
    beta = np.asarray(inputs["beta"], np.float32)
    W_ih = np.asarray(inputs["W_ih"], np.float32)
    b_ih = np.asarray(inputs["b_ih"], np.float32)
    W_hh = np.asarray(inputs["W_hh"], np.float32)
    b_hh = np.asarray(inputs["b_hh"], np.float32)
    W1 = np.asarray(inputs["W1"], np.float32)
    b1 = np.asarray(inputs["b1"], np.float32)
    W2 = np.asarray(inputs["W2"], np.float32)
    b2 = np.asarray(inputs["b2"], np.float32)
    num_actions = int(np.asarray(inputs["num_actions"]))
    assert num_actions == T, f"kernel hardcodes T={T}, got {num_actions}"
    assert obs.shape == (B, R)

    if "nc" not in _CACHE:
        _CACHE["nc"] = _build()
    nc = _CACHE["nc"]

    wtr_p = np.zeros((RP, Fd), np.float32)
    wtr_p[:R] = W_trunk
    wtr_p[R] = b_trunk          # bias via the first zero-padding row
    wtr = np.ascontiguousarray(
        wtr_p.reshape(NK, 128, Fd).transpose(1, 0, 2)).astype(F16H)
    wih = np.ascontiguousarray(
        W_ih.astype(F16H).reshape(KH, 128, M4, 128).transpose(2, 1, 0, 3)
    ).reshape(M4, 128, KH * 128)
    whh = np.ascontiguousarray(
        W_hh.reshape(KH, 128, 4 * H).transpose(1, 0, 2)).astype(F16H)
    w1 = np.ascontiguousarray(
        W1.reshape(KH, 128, H2).transpose(1, 0, 2)).astype(F16H)
    w2 = np.ascontiguousarray(
        W2.reshape(NH2, 128, A).transpose(1, 0, 2)).astype(F16H)
    # [128, M4] with bsum_sb[p, m] = (b_ih+b_hh)[m*128+p]: contiguous DMA lines
    bsum = np.ascontiguousarray(
        (b_ih + b_hh).astype(np.float32).reshape(M4, 128).T)
    b1h = np.ascontiguousarray(b1.astype(np.float32).reshape(NH2, 128).T)

    in_maps = []
    for i in range(NC_):
        sh = obs[i * BS : (i + 1) * BS]           # [256, R]
        obsT_p = np.zeros((RP, BS), np.float32)
        obsT_p[:R] = sh.T
        obsT_p[R] = 1.0         # pairs with the bias row of wtr
        obsT = np.ascontiguousarray(
            obsT_p.reshape(NK, 128, BS).transpose(1, 0, 2)).astype(F16H)
        in_maps.append({
            "obsT": obsT, "wtr": wtr, "wih": wih, "whh": whh,
            "w1": w1, "w2": w2, "gam": gamma,
            "bet": beta, "bsum": bsum, "b1": b1h, "b2": b2,
        })

    res = bass_utils.run_bass_kernel_spmd(
        nc, in_maps, core_ids=list(range(NC_)),
        trace=bool(int(__import__("os").environ.get("KTRACE", "0"))),
    )
    _CACHE["last_result"] = res
    out = np.concatenate(
        [res.results[i]["mu"].reshape(A, T, BS).transpose(2, 1, 0)
         for i in range(NC_)], axis=0
    )
    return np.ascontiguousarray(out)
